# revision 1
# baseline (speedup 1.0000x reference)
"""EpisodicGRU Trainium2 kernel.

Data-parallel over batch: 8 sequences per NeuronCore on 8 cores.
Per core the time recurrence is serial; per step we do
    psum_r = gi_r(t) + W_hr h   (gi precomputed by a big matmul, psum
    psum_z = -(gi_z(t) + W_hz h) initialized via identity-matmul copy)
    psum_n = b_hhn + W_hn h
    r = sigmoid(psum_r); sz = sigmoid(psum_z)          # sz = 1-z
    n = tanh(gi_n(t) + r * psum_n)
    zc = mwneg(t) * sz                                 # -m*w*(1-z)
    h = h*(1+zc) - zc*n
The input-gate GEMM for chunk c+1 is interleaved into the PE idle gaps
of chunk c's recurrence steps.
"""

import os
import sys

for _p in ("/opt/trn_rl_repo", "/root/.axon_site/_ro/trn_rl_repo",
           "/root/.axon_site", "/root/.axon_site/_ro/pypackages"):
    if os.path.isdir(_p) and _p not in sys.path:
        sys.path.append(_p)

import numpy as np
import ml_dtypes

import concourse.bass as bass
import concourse.bacc as bacc
import concourse.tile as tile
from concourse import mybir
from concourse.bass_utils import run_bass_kernel_spmd

F32 = mybir.dt.float32
F32R = mybir.dt.float32r
BF16 = mybir.dt.bfloat16
AF = mybir.ActivationFunctionType
BF16NP = ml_dtypes.bfloat16

B, T_FULL, I, H = 64, 2048, 256, 256
NCORES = 8
BS = B // NCORES          # 8 sequences per core
CH = 128                  # recurrence steps per chunk
GCOLS = BS                # 8 columns per (gate-half)
SCOLS = 2 * GCOLS         # 16 cols per step per gate (2 H-chunks)


def build_nc(T):
    nch = T // CH
    assert T % (2 * CH) == 0
    xflat = (T + 2 * CH) * BS          # padded flat (t, b) length
    mwflat = (T + 2 * CH) * SCOLS

    nc = bacc.Bacc("TRN2", target_bir_lowering=False, debug=False)

    xt_d = nc.dram_tensor("xt", [2, 128, xflat], BF16, kind="ExternalInput").ap()
    mw_d = nc.dram_tensor("mw", [1, mwflat], F32, kind="ExternalInput").ap()
    whT_d = nc.dram_tensor("whT", [12, 128, 128], BF16, kind="ExternalInput").ap()
    wiT_d = nc.dram_tensor("wiT", [12, 128, 128], BF16, kind="ExternalInput").ap()
    brz_d = nc.dram_tensor("brz", [128, 4], F32, kind="ExternalInput").ap()
    bn_d = nc.dram_tensor("bn", [128, 2], F32, kind="ExternalInput").ap()
    bhhn_d = nc.dram_tensor("bhhn", [128, SCOLS], F32R, kind="ExternalInput").ap()
    id_d = nc.dram_tensor("ident", [128, 128], F32R, kind="ExternalInput").ap()
    hout_d = nc.dram_tensor("hout", [128, SCOLS], F32, kind="ExternalOutput").ap()

    with tile.TileContext(nc) as tc:
        consts = tc.alloc_tile_pool(name="consts", bufs=1)
        state = tc.alloc_tile_pool(name="state", bufs=1)
        chunks = tc.alloc_tile_pool(name="chunks", bufs=1)
        temps = tc.alloc_tile_pool(name="temps", bufs=3)
        ps_r_pool = tc.alloc_tile_pool(name="psr", bufs=2, space="PSUM")
        ps_z_pool = tc.alloc_tile_pool(name="psz", bufs=2, space="PSUM")
        ps_n_pool = tc.alloc_tile_pool(name="psn", bufs=2, space="PSUM")
        ps_gi_pool = tc.alloc_tile_pool(name="psgi", bufs=2, space="PSUM")

        # ---- static tiles ----
        whT_s = consts.tile([128, 12 * 128], BF16, tag="whT")
        wiT_s = consts.tile([128, 12 * 128], BF16, tag="wiT")
        brz_s = consts.tile([128, 4], F32, tag="brz")
        bn_s = consts.tile([128, 2], F32, tag="bn")
        bhhn_s = consts.tile([128, SCOLS], F32R, tag="bhhn")
        id_s = consts.tile([128, 128], F32R, tag="ident")
        for t12 in range(12):
            nc.sync.dma_start(whT_s[:, t12 * 128:(t12 + 1) * 128], whT_d[t12])
            nc.sync.dma_start(wiT_s[:, t12 * 128:(t12 + 1) * 128], wiT_d[t12])
        nc.sync.dma_start(brz_s[:], brz_d[:])
        nc.sync.dma_start(bn_s[:], bn_d[:])
        nc.sync.dma_start(bhhn_s[:], bhhn_d[:])
        nc.sync.dma_start(id_s[:], id_d[:])

        h_f = state.tile([128, SCOLS], F32, tag="hf")
        h_b = state.tile([128, SCOLS], BF16, tag="hb")
        nc.vector.memset(h_f[:], 0.0)
        nc.vector.memset(h_b[:], 0.0)

        # ---- per-parity chunk buffers ----
        gi_rz = [chunks.tile([128, CH * 32], F32R, tag=f"girz{p}", name=f"girz{p}")
                 for p in range(2)]
        gin = [chunks.tile([128, CH * SCOLS], F32, tag=f"gin{p}", name=f"gin{p}")
               for p in range(2)]
        mw_s = [chunks.tile([128, CH * SCOLS], F32, tag=f"mw{p}", name=f"mw{p}")
                for p in range(2)]
        xs = [chunks.tile([128, 2 * CH * BS], BF16, tag=f"xs{p}", name=f"xs{p}")
              for p in range(2)]

        def dma_x(par, off_elems):
            # off_elems: flat (t,b) element offset of the chunk
            for half in range(2):
                nc.sync.dma_start(
                    xs[par][:, half * CH * BS:(half + 1) * CH * BS],
                    xt_d[half][:, bass.ds(off_elems, CH * BS)])

        def dma_mw(par, off_elems):
            nc.sync.dma_start(
                mw_s[par][:],
                mw_d[0:1, bass.ds(off_elems, CH * SCOLS)].partition_broadcast(128))

        # Work items producing gi for the chunk living in parity `par`,
        # consuming x from parity `par`.  Returns a list of ("mm"|"cp", thunk)
        # items to interleave into the recurrence steps: one matmul pair or
        # one psum->sbuf copy piece per item.
        def gi_items(par):
            items = []
            for half_n in range(2):        # N-tiles of 512 = 64 steps
                for j in range(6):
                    pg_box = []

                    def mk_mm(jj, nt, box):
                        def emit():
                            pg = ps_gi_pool.tile([128, 512], F32, tag="psgi",
                                                 name="psgi")
                            box.append(pg)
                            for k in range(2):
                                nc.tensor.matmul(
                                    pg[:],
                                    wiT_s[:, (k * 6 + jj) * 128:(k * 6 + jj + 1) * 128],
                                    xs[par][:, k * CH * BS + nt * 512:
                                            k * CH * BS + nt * 512 + 512],
                                    start=(k == 0), stop=(k == 1),
                                    skip_group_check=True)
                        return emit

                    def mk_cp(jj, nt, seg, box):
                        def emit():
                            pg = box[0]
                            pg3 = pg[:].rearrange("p (s b) -> p s b", b=GCOLS)
                            src = pg3[:, seg * 16:(seg + 1) * 16, :]
                            if jj < 4:     # r0,r1,z0,z1 -> gi_rz (bf16)
                                dst = gi_rz[par][:].rearrange(
                                    "p (s g) -> p s g", g=32)[
                                    :, nt * 64 + seg * 16:nt * 64 + (seg + 1) * 16,
                                    jj * GCOLS:(jj + 1) * GCOLS]
                                scale = 1.0 if jj < 2 else -1.0
                                nc.scalar.activation(
                                    dst, src, AF.Identity,
                                    bias=brz_s[:, jj:jj + 1], scale=scale)
                            else:          # n0,n1 -> gin (fp32)
                                jn = jj - 4
                                dst = gin[par][:].rearrange(
                                    "p (s g) -> p s g", g=SCOLS)[
                                    :, nt * 64 + seg * 16:nt * 64 + (seg + 1) * 16,
                                    jn * GCOLS:(jn + 1) * GCOLS]
                                nc.scalar.activation(
                                    dst, src, AF.Identity,
                                    bias=bn_s[:, jn:jn + 1], scale=1.0)
                        return emit

                    items.append(("mm", mk_mm(j, half_n, pg_box)))
                    for seg in range(4):
                        items.append(("cp", mk_cp(j, half_n, seg, pg_box)))
            return items

        W = whT_s

        def emit_step(par, s, mm_item, cp_item):
            ps_r = ps_r_pool.tile([128, SCOLS], F32, tag="psr")
            ps_z = ps_z_pool.tile([128, SCOLS], F32, tag="psz")
            ps_n = ps_n_pool.tile([128, SCOLS], F32, tag="psn")
            # PSUM init via identity matmul (sets has_written for accumulation).
            # float32r operands -> single-pass fp32 matmul (no LOW/HIGH split).
            nc.tensor.matmul(ps_r[:], id_s[:],
                             gi_rz[par][:, s * 32:s * 32 + 16],
                             start=True, stop=False, skip_group_check=True)
            nc.tensor.matmul(ps_z[:], id_s[:],
                             gi_rz[par][:, s * 32 + 16:s * 32 + 32],
                             start=True, stop=False, skip_group_check=True)
            nc.tensor.matmul(ps_n[:], id_s[:], bhhn_s[:],
                             start=True, stop=False, skip_group_check=True)
            # recurrent matmuls; r first (longest downstream chain), z, n
            for (j, dst) in ((0, ps_r), (1, ps_r), (2, ps_z), (3, ps_z),
                             (4, ps_n), (5, ps_n)):
                jj = j % 2
                for k in range(2):
                    nc.tensor.matmul(
                        dst[:, jj * GCOLS:(jj + 1) * GCOLS],
                        W[:, (k * 6 + j) * 128:(k * 6 + j + 1) * 128],
                        h_b[:, k * GCOLS:(k + 1) * GCOLS],
                        start=False, stop=(k == 1 and j in (1, 3, 5)),
                        skip_group_check=True)
            if mm_item:
                mm_item()
            sig_r = temps.tile([128, SCOLS], F32, tag="sigr")
            sig_z = temps.tile([128, SCOLS], F32, tag="sigz")
            t1 = temps.tile([128, SCOLS], F32, tag="t1")
            t2 = temps.tile([128, SCOLS], F32, tag="t2")
            n_t = temps.tile([128, SCOLS], F32, tag="nt")
            zc = temps.tile([128, SCOLS], F32, tag="zc")
            hm = temps.tile([128, SCOLS], F32, tag="hm")
            hzt = temps.tile([128, SCOLS], F32, tag="hzt")
            vv = temps.tile([128, SCOLS], F32, tag="vv")
            ww = temps.tile([128, SCOLS], F32, tag="ww")
            mwt = mw_s[par][:, s * SCOLS:(s + 1) * SCOLS]
            # hm = h*(-m*w): ready at step start, independent of this step's MMs
            nc.vector.tensor_mul(hm[:], h_f[:], mwt)
            nc.scalar.activation(sig_r[:], ps_r[:], AF.Sigmoid)
            nc.scalar.activation(sig_z[:], ps_z[:], AF.Sigmoid)
            # main chain: t1 -> t2 -> tanh -> w -> h
            nc.vector.tensor_mul(t1[:], sig_r[:], ps_n[:])
            nc.vector.tensor_add(t2[:], t1[:], gin[par][:, s * SCOLS:(s + 1) * SCOLS])
            nc.scalar.activation(n_t[:], t2[:], AF.Tanh)
            if cp_item:
                cp_item()
            # z branch (off critical path): v = h*(1-g) = h + (h*(-mw))*sz
            nc.vector.tensor_mul(zc[:], sig_z[:], mwt)
            nc.vector.tensor_mul(hzt[:], hm[:], sig_z[:])
            nc.vector.tensor_add(vv[:], h_f[:], hzt[:])
            # tail: h_new = v - zc*n
            nc.vector.tensor_mul(ww[:], zc[:], n_t[:])
            nc.vector.tensor_sub(h_b[:], vv[:], ww[:])
            nc.vector.tensor_sub(h_f[:], vv[:], ww[:])

        def emit_chunk(par, items):
            mm_q = [th for kind, th in items if kind == "mm"]
            cp_q = [th for kind, th in items if kind == "cp"]
            # order guarantee: copies of mm k must be emitted before mm k+2
            # (psum pool bufs=2).  mm at every 10th step, copies at every
            # other step keeps that order comfortably.
            mi = ci = 0
            for s in range(CH):
                mm_item = None
                cp_item = None
                if s % 10 == 1 and mi < len(mm_q):
                    mm_item = mm_q[mi]
                    mi += 1
                if s % 2 == 0 and ci < len(cp_q) and ci < 4 * mi:
                    cp_item = cp_q[ci]
                    ci += 1
                emit_step(par, s, mm_item, cp_item)
            while mi < len(mm_q):
                mm_q[mi]()
                mi += 1
            while ci < len(cp_q):
                cp_q[ci]()
                ci += 1

        # ---- prologue: x/mw for chunks 0,1 and gi for chunk 0 ----
        dma_x(0, 0)
        dma_x(1, CH * BS)
        dma_mw(0, 0)
        for _kind, th in gi_items(0):
            th()

        # ---- main loop over chunk pairs ----
        if nch > 2:
            assert nch % 2 == 0
            for j in range(nch // 2):
                # xs0 and mw_s1 are free at body start; xs1/mw_s0 are still
                # read during chunk A, so their refills are emitted after it.
                dma_x(0, j * (2 * CH * BS) + 2 * CH * BS)
                dma_mw(1, j * (2 * CH * SCOLS) + CH * SCOLS)
                emit_chunk(0, gi_items(1))
                dma_x(1, j * (2 * CH * BS) + 3 * CH * BS)
                dma_mw(0, j * (2 * CH * SCOLS) + 2 * CH * SCOLS)
                emit_chunk(1, gi_items(0))
        else:
            dma_mw(1, CH * SCOLS)
            emit_chunk(0, gi_items(1))
            emit_chunk(1, [])

        nc.sync.dma_start(hout_d[:], h_f[:])

        for p in (ps_gi_pool, ps_n_pool, ps_z_pool, ps_r_pool, temps,
                  chunks, state, consts):
            p.release()

    nc.compile()
    return nc


def host_prep(x, att_weights, lengths, W_ih, W_hh, b_ih, b_hh, T):
    """Build per-core input maps."""
    xpad = (T + 2 * CH)
    mask = (np.arange(T)[None, :] < np.asarray(lengths)[:, None])
    mwneg = (-(mask * np.asarray(att_weights)[:, :T])).astype(np.float32)  # [B,T]

    Wmod = np.concatenate([W_hh[0:H], -W_hh[H:2 * H], W_hh[2 * H:3 * H]], axis=0)
    whT = np.zeros((12, 128, 128), np.float32)
    wiT = np.zeros((12, 128, 128), np.float32)
    for k in range(2):
        for j in range(6):
            whT[k * 6 + j] = Wmod[j * 128:(j + 1) * 128, k * 128:(k + 1) * 128].T
            wiT[k * 6 + j] = W_ih[j * 128:(j + 1) * 128, k * 128:(k + 1) * 128].T
    whT = whT.astype(BF16NP)
    wiT = wiT.astype(BF16NP)

    bsum = (b_ih + b_hh).astype(np.float32)
    brz = np.zeros((128, 4), np.float32)
    brz[:, 0] = bsum[0:128]
    brz[:, 1] = bsum[128:256]
    brz[:, 2] = -bsum[256:384]
    brz[:, 3] = -bsum[384:512]
    bn = np.zeros((128, 2), np.float32)
    bn[:, 0] = b_ih[512:640]
    bn[:, 1] = b_ih[640:768]
    bhhn = np.zeros((128, SCOLS), np.float32)
    bhhn[:, 0:GCOLS] = np.repeat(b_hh[512:640][:, None], GCOLS, axis=1)
    bhhn[:, GCOLS:SCOLS] = np.repeat(b_hh[640:768][:, None], GCOLS, axis=1)
    ident = np.eye(128, dtype=np.float32)

    in_maps = []
    for c in range(NCORES):
        bs = slice(c * BS, (c + 1) * BS)
        xc = np.asarray(x[bs, :T]).transpose(2, 1, 0)       # [I, T, BS]
        xt = np.zeros((2, 128, xpad * BS), BF16NP)
        xt[:, :, :T * BS] = xc.reshape(2, 128, T * BS).astype(BF16NP)
        mwc = mwneg[bs].T                                    # [T, BS]
        mwt = np.zeros((1, xpad * SCOLS), np.float32)
        mwt[0, :T * SCOLS] = np.concatenate([mwc, mwc], axis=1).reshape(-1)
        in_maps.append({
            "xt": xt, "mw": mwt, "whT": whT, "wiT": wiT,
            "brz": brz, "bn": bn, "bhhn": bhhn, "ident": ident,
        })
    return in_maps


def assemble_out(results):
    out = np.zeros((B, H), np.float32)
    for c, res in enumerate(results):
        ho = res["hout"]                      # [128, 16]
        for k in range(2):
            out[c * BS:(c + 1) * BS, k * 128:(k + 1) * 128] = \
                ho[:, k * GCOLS:(k + 1) * GCOLS].T
    return out


def kernel(x, att_weights, lengths, W_ih, W_hh, b_ih, b_hh):
    x = np.asarray(x)
    in_maps = host_prep(np.asarray(x), np.asarray(att_weights),
                        np.asarray(lengths), np.asarray(W_ih),
                        np.asarray(W_hh), np.asarray(b_ih),
                        np.asarray(b_hh), T_FULL)
    nc = build_nc(T_FULL)
    res = None
    for attempt in range(3):
        try:
            res = run_bass_kernel_spmd(nc, in_maps, core_ids=list(range(NCORES)))
            break
        except Exception:
            if attempt == 2:
                raise
    return assemble_out(res.results)



# revision 2
# speedup vs baseline: 1.1676x; 1.1676x over previous
"""EpisodicGRU Trainium2 kernel.

Data-parallel over batch: 8 sequences per NeuronCore on 8 cores.
Per core the time recurrence is serial.  Let g = w*m*(1-z), then
    h' = h + g*(n - h) = (1+u)*h - u*n   with u = -g.
We keep v = (1+u)*h and q = u*n as separate tensors and let the PE
compute W*h = W*v - W*q by PSUM accumulation (whT for v, -whT for q),
so the critical chain per step is only
    q -> [4 r-matmuls] -> sigmoid_r -> t1 -> t2 -> tanh -> q
while v (available right after sigmoid_z) and all bookkeeping run off
the chain.  PSUM init: gi_rz via identity matmul; ps_n's b_hh bias via
a constant matmul (bconst rows 0/1 x selector).  The input-gate GEMM
for chunk c+1 is interleaved into chunk c's steps.  T is truncated at
runtime to max(lengths) rounded up to the chunk size.
"""

import os
import sys

for _p in ("/opt/trn_rl_repo", "/root/.axon_site/_ro/trn_rl_repo",
           "/root/.axon_site", "/root/.axon_site/_ro/pypackages"):
    if os.path.isdir(_p) and _p not in sys.path:
        sys.path.append(_p)

import numpy as np
import ml_dtypes

import concourse.bass as bass
import concourse.bacc as bacc
import concourse.tile as tile
from concourse import mybir
from concourse.bass_utils import run_bass_kernel_spmd

F32 = mybir.dt.float32
F32R = mybir.dt.float32r
BF16 = mybir.dt.bfloat16
AF = mybir.ActivationFunctionType
ALU = mybir.AluOpType
BF16NP = ml_dtypes.bfloat16

B, T_FULL, I, H = 64, 2048, 256, 256
NCORES = 8
BS = B // NCORES          # 8 sequences per core
CH = 128                  # recurrence steps per chunk
GCOLS = BS                # 8 columns per (gate-half)
SCOLS = 2 * GCOLS         # 16 cols per step per gate (2 H-chunks)


def plan_T(lengths):
    """Smallest multiple of CH (>= 2*CH) covering max(lengths)."""
    ml = int(np.max(np.asarray(lengths)))
    nch = max(2, -(-ml // CH))
    return nch * CH


def build_nc(T):
    nch = T // CH
    assert T % CH == 0 and nch >= 2
    xflat = (T + 2 * CH) * BS          # padded flat (t, b) length
    mwflat = (T + 2 * CH) * SCOLS

    nc = bacc.Bacc("TRN2", target_bir_lowering=False, debug=False)

    xt_d = nc.dram_tensor("xt", [2, 128, xflat], BF16, kind="ExternalInput").ap()
    mw_d = nc.dram_tensor("mw", [1, mwflat], F32, kind="ExternalInput").ap()
    whT_d = nc.dram_tensor("whT", [12, 128, 128], BF16, kind="ExternalInput").ap()
    whTn_d = nc.dram_tensor("whTn", [12, 128, 128], BF16, kind="ExternalInput").ap()
    wiT_d = nc.dram_tensor("wiT", [12, 128, 128], BF16, kind="ExternalInput").ap()
    brz_d = nc.dram_tensor("brz", [128, 4], F32, kind="ExternalInput").ap()
    bn_d = nc.dram_tensor("bn", [128, 2], F32, kind="ExternalInput").ap()
    bconst_d = nc.dram_tensor("bconst", [128, 128], BF16, kind="ExternalInput").ap()
    sel_d = nc.dram_tensor("sel", [128, SCOLS], BF16, kind="ExternalInput").ap()
    id_d = nc.dram_tensor("ident", [128, 128], F32R, kind="ExternalInput").ap()
    hout_d = nc.dram_tensor("hout", [128, SCOLS], F32, kind="ExternalOutput").ap()

    with tile.TileContext(nc) as tc:
        consts = tc.alloc_tile_pool(name="consts", bufs=1)
        state = tc.alloc_tile_pool(name="state", bufs=1)
        chunks = tc.alloc_tile_pool(name="chunks", bufs=1)
        temps = tc.alloc_tile_pool(name="temps", bufs=3)
        ps_r_pool = tc.alloc_tile_pool(name="psr", bufs=2, space="PSUM")
        ps_z_pool = tc.alloc_tile_pool(name="psz", bufs=2, space="PSUM")
        ps_n_pool = tc.alloc_tile_pool(name="psn", bufs=2, space="PSUM")
        ps_gi_pool = tc.alloc_tile_pool(name="psgi", bufs=2, space="PSUM")

        # ---- static tiles ----
        whT_s = consts.tile([128, 12 * 128], BF16, tag="whT")
        whTn_s = consts.tile([128, 12 * 128], BF16, tag="whTn")
        wiT_s = consts.tile([128, 12 * 128], BF16, tag="wiT")
        brz_s = consts.tile([128, 4], F32, tag="brz")
        bn_s = consts.tile([128, 2], F32, tag="bn")
        bconst_s = consts.tile([128, 128], BF16, tag="bconst")
        sel_s = consts.tile([128, SCOLS], BF16, tag="sel")
        id_s = consts.tile([128, 128], F32R, tag="ident")
        for t12 in range(12):
            nc.sync.dma_start(whT_s[:, t12 * 128:(t12 + 1) * 128], whT_d[t12])
            nc.sync.dma_start(whTn_s[:, t12 * 128:(t12 + 1) * 128], whTn_d[t12])
            nc.sync.dma_start(wiT_s[:, t12 * 128:(t12 + 1) * 128], wiT_d[t12])
        nc.sync.dma_start(brz_s[:], brz_d[:])
        nc.sync.dma_start(bn_s[:], bn_d[:])
        nc.sync.dma_start(bconst_s[:], bconst_d[:])
        nc.sync.dma_start(sel_s[:], sel_d[:])
        nc.sync.dma_start(id_s[:], id_d[:])

        h_f = state.tile([128, SCOLS], F32, tag="hf")
        v_b = state.tile([128, SCOLS], BF16, tag="vb")
        q_b = state.tile([128, SCOLS], BF16, tag="qb")
        v_f = state.tile([128, SCOLS], F32, tag="vf")
        nc.vector.memset(h_f[:], 0.0)
        nc.vector.memset(v_b[:], 0.0)
        nc.vector.memset(q_b[:], 0.0)
        nc.vector.memset(v_f[:], 0.0)

        # ---- per-parity chunk buffers ----
        gi_rz = [chunks.tile([128, CH * 32], F32R, tag=f"girz{p}", name=f"girz{p}")
                 for p in range(2)]
        gin = [chunks.tile([128, CH * SCOLS], F32, tag=f"gin{p}", name=f"gin{p}")
               for p in range(2)]
        mw_s = [chunks.tile([128, CH * SCOLS], F32, tag=f"mw{p}", name=f"mw{p}")
                for p in range(2)]
        xs = [chunks.tile([128, 2 * CH * BS], BF16, tag=f"xs{p}", name=f"xs{p}")
              for p in range(2)]

        def dma_x(par, off_elems):
            # off_elems: flat (t,b) element offset of the chunk
            for half in range(2):
                nc.sync.dma_start(
                    xs[par][:, half * CH * BS:(half + 1) * CH * BS],
                    xt_d[half][:, bass.ds(off_elems, CH * BS)])

        def dma_mw(par, off_elems):
            nc.sync.dma_start(
                mw_s[par][:],
                mw_d[0:1, bass.ds(off_elems, CH * SCOLS)].partition_broadcast(128))

        # Work items producing gi for the chunk living in parity `par`,
        # consuming x from parity `par`.  Returns a list of ("mm"|"cp", thunk)
        # items to interleave into the recurrence steps: one matmul pair or
        # one psum->sbuf copy piece per item.
        def gi_items(par):
            items = []
            for half_n in range(2):        # N-tiles of 512 = 64 steps
                for j in range(6):
                    pg_box = []

                    def mk_mm(jj, nt, box):
                        def emit():
                            pg = ps_gi_pool.tile([128, 512], F32, tag="psgi",
                                                 name="psgi")
                            box.append(pg)
                            for k in range(2):
                                nc.tensor.matmul(
                                    pg[:],
                                    wiT_s[:, (k * 6 + jj) * 128:(k * 6 + jj + 1) * 128],
                                    xs[par][:, k * CH * BS + nt * 512:
                                            k * CH * BS + nt * 512 + 512],
                                    start=(k == 0), stop=(k == 1),
                                    skip_group_check=True)
                        return emit

                    def mk_cp(jj, nt, seg, box):
                        def emit():
                            pg = box[0]
                            pg3 = pg[:].rearrange("p (s b) -> p s b", b=GCOLS)
                            src = pg3[:, seg * 16:(seg + 1) * 16, :]
                            if jj < 4:     # r0,r1,z0,z1 -> gi_rz
                                dst = gi_rz[par][:].rearrange(
                                    "p (s g) -> p s g", g=32)[
                                    :, nt * 64 + seg * 16:nt * 64 + (seg + 1) * 16,
                                    jj * GCOLS:(jj + 1) * GCOLS]
                                scale = 1.0 if jj < 2 else -1.0
                                nc.scalar.activation(
                                    dst, src, AF.Identity,
                                    bias=brz_s[:, jj:jj + 1], scale=scale)
                            else:          # n0,n1 -> gin (fp32)
                                jn = jj - 4
                                dst = gin[par][:].rearrange(
                                    "p (s g) -> p s g", g=SCOLS)[
                                    :, nt * 64 + seg * 16:nt * 64 + (seg + 1) * 16,
                                    jn * GCOLS:(jn + 1) * GCOLS]
                                nc.scalar.activation(
                                    dst, src, AF.Identity,
                                    bias=bn_s[:, jn:jn + 1], scale=1.0)
                        return emit

                    items.append(("mm", mk_mm(j, half_n, pg_box)))
                    for seg in range(4):
                        items.append(("cp", mk_cp(j, half_n, seg, pg_box)))
            return items

        W = whT_s
        Wn = whTn_s

        def emit_step(par, s, mm_item, cp_item):
            ps_r = ps_r_pool.tile([128, SCOLS], F32, tag="psr")
            ps_z = ps_z_pool.tile([128, SCOLS], F32, tag="psz")
            ps_n = ps_n_pool.tile([128, SCOLS], F32, tag="psn")
            # PSUM init (sets has_written for accumulation): gi for r/z via
            # identity matmul; b_hhn for n via constant matmul.
            nc.tensor.matmul(ps_r[:], id_s[:],
                             gi_rz[par][:, s * 32:s * 32 + 16],
                             start=True, stop=False, skip_group_check=True)
            nc.tensor.matmul(ps_z[:], id_s[:],
                             gi_rz[par][:, s * 32 + 16:s * 32 + 32],
                             start=True, stop=False, skip_group_check=True)
            nc.tensor.matmul(ps_n[:], bconst_s[:], sel_s[:],
                             start=True, stop=False, skip_group_check=True)
            # W*h = W*v - W*q; v-matmuls first (v ready early), then q.
            for (j, dst) in ((0, ps_r), (1, ps_r), (2, ps_z), (3, ps_z),
                             (4, ps_n), (5, ps_n)):
                jj = j % 2
                for k in range(2):
                    nc.tensor.matmul(
                        dst[:, jj * GCOLS:(jj + 1) * GCOLS],
                        W[:, (k * 6 + j) * 128:(k * 6 + j + 1) * 128],
                        v_b[:, k * GCOLS:(k + 1) * GCOLS],
                        start=False, stop=False, skip_group_check=True)
            for (j, dst) in ((0, ps_r), (1, ps_r), (2, ps_z), (3, ps_z),
                             (4, ps_n), (5, ps_n)):
                jj = j % 2
                for k in range(2):
                    nc.tensor.matmul(
                        dst[:, jj * GCOLS:(jj + 1) * GCOLS],
                        Wn[:, (k * 6 + j) * 128:(k * 6 + j + 1) * 128],
                        q_b[:, k * GCOLS:(k + 1) * GCOLS],
                        start=False, stop=(k == 1 and j in (1, 3, 5)),
                        skip_group_check=True)
            if mm_item:
                mm_item()
            sig_r = temps.tile([128, SCOLS], F32, tag="sigr")
            sig_z = temps.tile([128, SCOLS], F32, tag="sigz")
            t1 = temps.tile([128, SCOLS], F32, tag="t1")
            t2 = temps.tile([128, SCOLS], F32, tag="t2")
            n_t = temps.tile([128, SCOLS], F32, tag="nt")
            u1 = temps.tile([128, SCOLS], F32, tag="u1")
            qq = temps.tile([128, SCOLS], F32, tag="qq")
            mwt = mw_s[par][:, s * SCOLS:(s + 1) * SCOLS]
            nc.scalar.activation(sig_r[:], ps_r[:], AF.Sigmoid)
            nc.scalar.activation(sig_z[:], ps_z[:], AF.Sigmoid)
            # chain: t1 -> t2 -> tanh -> q_b
            nc.vector.tensor_mul(t1[:], sig_r[:], ps_n[:])
            nc.vector.tensor_add(t2[:], t1[:], gin[par][:, s * SCOLS:(s + 1) * SCOLS])
            # off-chain: u = -g, v = (1+u)*h (bf16 twin for the PE)
            nc.vector.tensor_mul(u1[:], sig_z[:], mwt)
            nc.vector.scalar_tensor_tensor(v_f[:], u1[:], 1.0, h_f[:],
                                           ALU.add, ALU.mult)
            nc.vector.scalar_tensor_tensor(v_b[:], u1[:], 1.0, h_f[:],
                                           ALU.add, ALU.mult)
            nc.scalar.activation(n_t[:], t2[:], AF.Tanh)
            if cp_item:
                cp_item()
            nc.vector.tensor_mul(q_b[:], u1[:], n_t[:])
            nc.vector.tensor_mul(qq[:], u1[:], n_t[:])
            nc.vector.tensor_sub(h_f[:], v_f[:], qq[:])

        def emit_chunk(par, items):
            mm_q = [th for kind, th in items if kind == "mm"]
            cp_q = [th for kind, th in items if kind == "cp"]
            # order guarantee: copies of mm k must be emitted before mm k+2
            # (psum pool bufs=2).  mm at every 10th step, copies at every
            # other step keeps that order comfortably.
            mi = ci = 0
            for s in range(CH):
                mm_item = None
                cp_item = None
                if s % 10 == 1 and mi < len(mm_q):
                    mm_item = mm_q[mi]
                    mi += 1
                if s % 2 == 0 and ci < len(cp_q) and ci < 4 * mi:
                    cp_item = cp_q[ci]
                    ci += 1
                emit_step(par, s, mm_item, cp_item)
            while mi < len(mm_q):
                mm_q[mi]()
                mi += 1
            while ci < len(cp_q):
                cp_q[ci]()
                ci += 1

        # ---- prologue: x/mw for chunks 0,1 and gi for chunk 0 ----
        dma_x(0, 0)
        dma_x(1, CH * BS)
        dma_mw(0, 0)
        for _kind, th in gi_items(0):
            th()

        # ---- main loop over chunks ----
        for c in range(nch):
            par = c % 2
            if c + 2 < nch:
                dma_x(par, (c + 2) * CH * BS)
            if c + 1 < nch:
                dma_mw(1 - par, (c + 1) * CH * SCOLS)
            emit_chunk(par, gi_items(1 - par) if c + 1 < nch else [])

        nc.sync.dma_start(hout_d[:], h_f[:])

        for p in (ps_gi_pool, ps_n_pool, ps_z_pool, ps_r_pool, temps,
                  chunks, state, consts):
            p.release()

    nc.compile()
    return nc


def host_prep(x, att_weights, lengths, W_ih, W_hh, b_ih, b_hh, T):
    """Build per-core input maps."""
    xpad = (T + 2 * CH)
    mask = (np.arange(T)[None, :] < np.asarray(lengths)[:, None])
    mwneg = (-(mask * np.asarray(att_weights)[:, :T])).astype(np.float32)  # [B,T]

    Wmod = np.concatenate([W_hh[0:H], -W_hh[H:2 * H], W_hh[2 * H:3 * H]], axis=0)
    whT = np.zeros((12, 128, 128), np.float32)
    wiT = np.zeros((12, 128, 128), np.float32)
    for k in range(2):
        for j in range(6):
            whT[k * 6 + j] = Wmod[j * 128:(j + 1) * 128, k * 128:(k + 1) * 128].T
            wiT[k * 6 + j] = W_ih[j * 128:(j + 1) * 128, k * 128:(k + 1) * 128].T
    whT = whT.astype(BF16NP)
    whTn = (-whT).astype(BF16NP)
    wiT = wiT.astype(BF16NP)

    bsum = (b_ih + b_hh).astype(np.float32)
    brz = np.zeros((128, 4), np.float32)
    brz[:, 0] = bsum[0:128]
    brz[:, 1] = bsum[128:256]
    brz[:, 2] = -bsum[256:384]
    brz[:, 3] = -bsum[384:512]
    bn = np.zeros((128, 2), np.float32)
    bn[:, 0] = b_ih[512:640]
    bn[:, 1] = b_ih[640:768]
    # ps_n bias matmul: out[f, c] = sum_p bconst[p, f] * sel[p, c]
    bconst = np.zeros((128, 128), np.float32)
    bconst[0, :] = b_hh[512:640]
    bconst[1, :] = b_hh[640:768]
    bconst = bconst.astype(BF16NP)
    sel = np.zeros((128, SCOLS), np.float32)
    sel[0, 0:GCOLS] = 1.0
    sel[1, GCOLS:SCOLS] = 1.0
    sel = sel.astype(BF16NP)
    ident = np.eye(128, dtype=np.float32)

    in_maps = []
    for c in range(NCORES):
        bs = slice(c * BS, (c + 1) * BS)
        xc = np.asarray(x[bs, :T]).transpose(2, 1, 0)       # [I, T, BS]
        xt = np.zeros((2, 128, xpad * BS), BF16NP)
        xt[:, :, :T * BS] = xc.reshape(2, 128, T * BS).astype(BF16NP)
        mwc = mwneg[bs].T                                    # [T, BS]
        mwt = np.zeros((1, xpad * SCOLS), np.float32)
        mwt[0, :T * SCOLS] = np.concatenate([mwc, mwc], axis=1).reshape(-1)
        in_maps.append({
            "xt": xt, "mw": mwt, "whT": whT, "whTn": whTn, "wiT": wiT,
            "brz": brz, "bn": bn, "bconst": bconst, "sel": sel, "ident": ident,
        })
    return in_maps


def assemble_out(results):
    out = np.zeros((B, H), np.float32)
    for c, res in enumerate(results):
        ho = res["hout"]                      # [128, 16]
        for k in range(2):
            out[c * BS:(c + 1) * BS, k * 128:(k + 1) * 128] = \
                ho[:, k * GCOLS:(k + 1) * GCOLS].T
    return out


def kernel(x, att_weights, lengths, W_ih, W_hh, b_ih, b_hh):
    x = np.asarray(x)
    lengths = np.asarray(lengths)
    T_run = plan_T(lengths)
    in_maps = host_prep(np.asarray(x), np.asarray(att_weights),
                        lengths, np.asarray(W_ih),
                        np.asarray(W_hh), np.asarray(b_ih),
                        np.asarray(b_hh), T_run)
    nc = build_nc(T_run)
    res = None
    for attempt in range(3):
        try:
            res = run_bass_kernel_spmd(nc, in_maps, core_ids=list(range(NCORES)))
            break
        except Exception:
            if attempt == 2:
                raise
    return assemble_out(res.results)


# revision 8
# speedup vs baseline: 1.3904x; 1.1908x over previous
"""EpisodicGRU Trainium2 kernel.

Data-parallel over batch: 8 sequences per NeuronCore on 8 cores.
Per core the time recurrence is serial.  Let g = w*m*(1-z), then
    h' = h + g*(n - h) = (1+u)*h - u*n   with u = -g.
We keep v = (1+u)*h and q = u*n as separate tensors and let the PE
compute W*h = W*v - W*q by PSUM accumulation (whT for v, -whT for q),
so the critical chain per step is only
    q -> [4 r-matmuls] -> sigmoid_r -> t1 -> t2 -> tanh -> q
while v (available right after sigmoid_z) and all bookkeeping run off
the chain.  PSUM init: gi_rz via identity matmul; ps_n's b_hh bias via
a constant matmul (bconst rows 0/1 x selector).  The input-gate GEMM
for chunk c+1 is interleaved into chunk c's steps.  T is truncated at
runtime to max(lengths) rounded up to the chunk size.
"""

import os
import sys

for _p in ("/opt/trn_rl_repo", "/root/.axon_site/_ro/trn_rl_repo",
           "/root/.axon_site", "/root/.axon_site/_ro/pypackages"):
    if os.path.isdir(_p) and _p not in sys.path:
        sys.path.append(_p)

import numpy as np
import ml_dtypes

import concourse.bass as bass
import concourse.bacc as bacc
import concourse.tile as tile
from concourse import mybir
from concourse.bass_utils import run_bass_kernel_spmd

F32 = mybir.dt.float32
F32R = mybir.dt.float32r
BF16 = mybir.dt.bfloat16
AF = mybir.ActivationFunctionType
ALU = mybir.AluOpType
BF16NP = ml_dtypes.bfloat16

B, T_FULL, I, H = 64, 2048, 256, 256
NCORES = 8
BS = B // NCORES          # 8 sequences per core
CH = 128                  # recurrence steps per chunk
GCOLS = BS                # 8 columns per (gate-half)
SCOLS = 2 * GCOLS         # 16 cols per step per gate (2 H-chunks)


def plan_T(lengths):
    """Smallest multiple of CH (>= 2*CH) covering max(lengths)."""
    ml = int(np.max(np.asarray(lengths)))
    nch = max(2, -(-ml // CH))
    return nch * CH


def build_nc(T):
    nch = T // CH
    assert T % CH == 0 and nch >= 2
    xflat = (T + 2 * CH) * BS          # padded flat (t, b) length
    mwflat = (T + 2 * CH) * SCOLS

    nc = bacc.Bacc("TRN2", target_bir_lowering=False, debug=False)

    xt_d = nc.dram_tensor("xt", [2, 128, xflat], BF16, kind="ExternalInput").ap()
    mw_d = nc.dram_tensor("mw", [1, mwflat], F32, kind="ExternalInput").ap()
    whT_d = nc.dram_tensor("whT", [12, 128, 128], BF16, kind="ExternalInput").ap()
    whTn_d = nc.dram_tensor("whTn", [12, 128, 128], BF16, kind="ExternalInput").ap()
    wiT_d = nc.dram_tensor("wiT", [12, 128, 128], BF16, kind="ExternalInput").ap()
    brz_d = nc.dram_tensor("brz", [128, 4], F32, kind="ExternalInput").ap()
    bn_d = nc.dram_tensor("bn", [128, 2], F32, kind="ExternalInput").ap()
    bconst_d = nc.dram_tensor("bconst", [128, 128], BF16, kind="ExternalInput").ap()
    sel_d = nc.dram_tensor("sel", [128, SCOLS], BF16, kind="ExternalInput").ap()
    id_d = nc.dram_tensor("ident", [128, 128], F32R, kind="ExternalInput").ap()
    hout_d = nc.dram_tensor("hout", [128, SCOLS], F32, kind="ExternalOutput").ap()

    with tile.TileContext(nc) as tc:
        consts = tc.alloc_tile_pool(name="consts", bufs=1)
        state = tc.alloc_tile_pool(name="state", bufs=1)
        chunks = tc.alloc_tile_pool(name="chunks", bufs=1)
        temps = tc.alloc_tile_pool(name="temps", bufs=3)
        ps_r_pool = tc.alloc_tile_pool(name="psr", bufs=2, space="PSUM")
        ps_z_pool = tc.alloc_tile_pool(name="psz", bufs=2, space="PSUM")
        ps_n_pool = tc.alloc_tile_pool(name="psn", bufs=2, space="PSUM")
        ps_gi_pool = tc.alloc_tile_pool(name="psgi", bufs=2, space="PSUM")

        # ---- static tiles ----
        whT_s = consts.tile([128, 12 * 128], BF16, tag="whT")
        whTn_s = consts.tile([128, 12 * 128], BF16, tag="whTn")
        wiT_s = consts.tile([128, 12 * 128], BF16, tag="wiT")
        brz_s = consts.tile([128, 4], F32, tag="brz")
        bn_s = consts.tile([128, 2], F32, tag="bn")
        bconst_s = consts.tile([128, 128], BF16, tag="bconst")
        sel_s = consts.tile([128, SCOLS], BF16, tag="sel")
        id_s = consts.tile([128, 128], F32R, tag="ident")
        for t12 in range(12):
            nc.sync.dma_start(whT_s[:, t12 * 128:(t12 + 1) * 128], whT_d[t12])
            nc.sync.dma_start(whTn_s[:, t12 * 128:(t12 + 1) * 128], whTn_d[t12])
            nc.sync.dma_start(wiT_s[:, t12 * 128:(t12 + 1) * 128], wiT_d[t12])
        nc.sync.dma_start(brz_s[:], brz_d[:])
        nc.sync.dma_start(bn_s[:], bn_d[:])
        nc.sync.dma_start(bconst_s[:], bconst_d[:])
        nc.sync.dma_start(sel_s[:], sel_d[:])
        nc.sync.dma_start(id_s[:], id_d[:])

        h_f = state.tile([128, SCOLS], F32, tag="hf")
        v_b = state.tile([128, SCOLS], BF16, tag="vb")
        q_b = state.tile([128, SCOLS], BF16, tag="qb")
        v_f = state.tile([128, SCOLS], F32, tag="vf")
        nc.vector.memset(h_f[:], 0.0)
        nc.vector.memset(v_b[:], 0.0)
        nc.vector.memset(q_b[:], 0.0)
        nc.vector.memset(v_f[:], 0.0)

        # ---- per-parity chunk buffers ----
        gi_rz = [chunks.tile([128, CH * 32], F32R, tag=f"girz{p}", name=f"girz{p}")
                 for p in range(2)]
        gin = [chunks.tile([128, CH * SCOLS], F32, tag=f"gin{p}", name=f"gin{p}")
               for p in range(2)]
        mw_s = [chunks.tile([128, CH * SCOLS], F32, tag=f"mw{p}", name=f"mw{p}")
                for p in range(2)]
        xs = [chunks.tile([128, 2 * CH * BS], BF16, tag=f"xs{p}", name=f"xs{p}")
              for p in range(2)]

        def dma_x(par, off_elems):
            # off_elems: flat (t,b) element offset of the chunk
            for half in range(2):
                nc.sync.dma_start(
                    xs[par][:, half * CH * BS:(half + 1) * CH * BS],
                    xt_d[half][:, bass.ds(off_elems, CH * BS)])

        def dma_mw(par, off_elems):
            nc.sync.dma_start(
                mw_s[par][:],
                mw_d[0:1, bass.ds(off_elems, CH * SCOLS)].partition_broadcast(128))

        # gi GEMM for the chunk living in parity `par`, consuming x from
        # parity `par`, finely sliced so that each injected PE matmul is
        # <=256 moving cols (~390ns) and each injected psum->sbuf copy is
        # <=64 cols (~250ns) -- small enough not to stall the recurrence
        # chain.  Returns {step: [("mm"|"cp", thunk), ...]}.
        def gi_sched(par):
            sched = {}
            for i in range(12):            # (half_n, j) items
                half_n, j = divmod(i, 6)
                s0 = 1 + (i * 115) // 12   # item start step; 4 mm + 8 cp slots
                box = []

                def mk_mm(jj, nt, ph, k, box):
                    def emit():
                        if not box:
                            box.append(ps_gi_pool.tile([128, 512], F32,
                                                       tag="psgi", name="psgi"))
                        pg = box[0]
                        nc.tensor.matmul(
                            pg[:, ph * 256:(ph + 1) * 256],
                            wiT_s[:, (k * 6 + jj) * 128:(k * 6 + jj + 1) * 128],
                            xs[par][:, k * CH * BS + nt * 512 + ph * 256:
                                    k * CH * BS + nt * 512 + ph * 256 + 256],
                            start=(k == 0), stop=(k == 1),
                            skip_group_check=True)
                    return emit

                def mk_cp(jj, nt, seg, box):
                    def emit():
                        pg = box[0]
                        pg3 = pg[:].rearrange("p (s b) -> p s b", b=GCOLS)
                        src = pg3[:, seg * 8:(seg + 1) * 8, :]
                        if jj < 4:     # r0,r1,z0,z1 -> gi_rz
                            dst = gi_rz[par][:].rearrange(
                                "p (s g) -> p s g", g=32)[
                                :, nt * 64 + seg * 8:nt * 64 + (seg + 1) * 8,
                                jj * GCOLS:(jj + 1) * GCOLS]
                            scale = 1.0 if jj < 2 else -1.0
                            nc.scalar.activation(
                                dst, src, AF.Identity,
                                bias=brz_s[:, jj:jj + 1], scale=scale)
                        else:          # n0,n1 -> gin (fp32)
                            jn = jj - 4
                            dst = gin[par][:].rearrange(
                                "p (s g) -> p s g", g=SCOLS)[
                                :, nt * 64 + seg * 8:nt * 64 + (seg + 1) * 8,
                                jn * GCOLS:(jn + 1) * GCOLS]
                            nc.scalar.activation(
                                dst, src, AF.Identity,
                                bias=bn_s[:, jn:jn + 1], scale=1.0)
                    return emit

                for idx, (ph, k) in enumerate(((0, 0), (0, 1), (1, 0), (1, 1))):
                    sched.setdefault(s0 + idx, []).append(
                        ("mm", mk_mm(j, half_n, ph, k, box)))
                for seg in range(8):
                    sched.setdefault(s0 + 4 + seg, []).append(
                        ("cp", mk_cp(j, half_n, seg, box)))
            return sched

        W = whT_s
        Wn = whTn_s

        def emit_step(par, s, mm_items, cp_items):
            ps_r = ps_r_pool.tile([128, SCOLS], F32, tag="psr")
            ps_z = ps_z_pool.tile([128, SCOLS], F32, tag="psz")
            ps_n = ps_n_pool.tile([128, SCOLS], F32, tag="psn")
            # PSUM init (sets has_written for accumulation): gi for r/z via
            # identity matmul; b_hhn for n via constant matmul.
            nc.tensor.matmul(ps_r[:], id_s[:],
                             gi_rz[par][:, s * 32:s * 32 + 16],
                             start=True, stop=False, skip_group_check=True)
            nc.tensor.matmul(ps_z[:], id_s[:],
                             gi_rz[par][:, s * 32 + 16:s * 32 + 32],
                             start=True, stop=False, skip_group_check=True)
            nc.tensor.matmul(ps_n[:], bconst_s[:], sel_s[:],
                             start=True, stop=False, skip_group_check=True)
            # W*h = W*v - W*q; v-matmuls first (v ready early), then q.
            for (j, dst) in ((0, ps_r), (1, ps_r), (2, ps_z), (3, ps_z),
                             (4, ps_n), (5, ps_n)):
                jj = j % 2
                for k in range(2):
                    nc.tensor.matmul(
                        dst[:, jj * GCOLS:(jj + 1) * GCOLS],
                        W[:, (k * 6 + j) * 128:(k * 6 + j + 1) * 128],
                        v_b[:, k * GCOLS:(k + 1) * GCOLS],
                        start=False, stop=False, skip_group_check=True)
            for (j, dst) in ((0, ps_r), (1, ps_r), (2, ps_z), (3, ps_z),
                             (4, ps_n), (5, ps_n)):
                jj = j % 2
                for k in range(2):
                    nc.tensor.matmul(
                        dst[:, jj * GCOLS:(jj + 1) * GCOLS],
                        Wn[:, (k * 6 + j) * 128:(k * 6 + j + 1) * 128],
                        q_b[:, k * GCOLS:(k + 1) * GCOLS],
                        start=False, stop=(k == 1 and j in (1, 3, 5)),
                        skip_group_check=True)
            for th in mm_items:
                th()
            sig_r = temps.tile([128, SCOLS], F32, tag="sigr")
            sig_z = temps.tile([128, SCOLS], F32, tag="sigz")
            t1 = temps.tile([128, SCOLS], F32, tag="t1")
            t2 = temps.tile([128, SCOLS], F32, tag="t2")
            n_t = temps.tile([128, SCOLS], F32, tag="nt")
            u1 = temps.tile([128, SCOLS], F32, tag="u1")
            qq = temps.tile([128, SCOLS], F32, tag="qq")
            mwt = mw_s[par][:, s * SCOLS:(s + 1) * SCOLS]
            nc.scalar.activation(sig_r[:], ps_r[:], AF.Sigmoid)
            nc.scalar.activation(sig_z[:], ps_z[:], AF.Sigmoid)
            # chain: t1 -> t2 -> tanh -> q_b
            nc.vector.tensor_mul(t1[:], sig_r[:], ps_n[:])
            nc.vector.tensor_add(t2[:], t1[:], gin[par][:, s * SCOLS:(s + 1) * SCOLS])
            # off-chain: u = -g, v = (1+u)*h (bf16 twin for the PE)
            nc.vector.tensor_mul(u1[:], sig_z[:], mwt)
            nc.vector.scalar_tensor_tensor(v_f[:], u1[:], 1.0, h_f[:],
                                           ALU.add, ALU.mult)
            nc.vector.scalar_tensor_tensor(v_b[:], u1[:], 1.0, h_f[:],
                                           ALU.add, ALU.mult)
            nc.scalar.activation(n_t[:], t2[:], AF.Tanh)
            for th in cp_items:
                th()
            nc.vector.tensor_mul(q_b[:], u1[:], n_t[:])
            nc.vector.tensor_mul(qq[:], u1[:], n_t[:])
            nc.vector.tensor_sub(h_f[:], v_f[:], qq[:])

        def emit_chunk(par, sched):
            for s in range(CH):
                items = sched.get(s, ())
                emit_step(par, s,
                          [th for kind, th in items if kind == "mm"],
                          [th for kind, th in items if kind == "cp"])
            for s in sorted(k for k in sched if k >= CH):
                for _kind, th in sched[s]:
                    th()

        # ---- prologue: x/mw for chunks 0,1 and gi for chunk 0 ----
        dma_x(0, 0)
        dma_x(1, CH * BS)
        dma_mw(0, 0)
        sched0 = gi_sched(0)
        for s in sorted(sched0):
            for _kind, th in sched0[s]:
                th()

        # ---- main loop over chunks ----
        for c in range(nch):
            par = c % 2
            if c + 2 < nch:
                dma_x(par, (c + 2) * CH * BS)
            if c + 1 < nch:
                dma_mw(1 - par, (c + 1) * CH * SCOLS)
            emit_chunk(par, gi_sched(1 - par) if c + 1 < nch else {})

        nc.sync.dma_start(hout_d[:], h_f[:])

        for p in (ps_gi_pool, ps_n_pool, ps_z_pool, ps_r_pool, temps,
                  chunks, state, consts):
            p.release()

    nc.compile()
    return nc


def host_prep(x, att_weights, lengths, W_ih, W_hh, b_ih, b_hh, T):
    """Build per-core input maps."""
    xpad = (T + 2 * CH)
    mask = (np.arange(T)[None, :] < np.asarray(lengths)[:, None])
    mwneg = (-(mask * np.asarray(att_weights)[:, :T])).astype(np.float32)  # [B,T]

    Wmod = np.concatenate([W_hh[0:H], -W_hh[H:2 * H], W_hh[2 * H:3 * H]], axis=0)
    whT = np.zeros((12, 128, 128), np.float32)
    wiT = np.zeros((12, 128, 128), np.float32)
    for k in range(2):
        for j in range(6):
            whT[k * 6 + j] = Wmod[j * 128:(j + 1) * 128, k * 128:(k + 1) * 128].T
            wiT[k * 6 + j] = W_ih[j * 128:(j + 1) * 128, k * 128:(k + 1) * 128].T
    whT = whT.astype(BF16NP)
    whTn = (-whT).astype(BF16NP)
    wiT = wiT.astype(BF16NP)

    bsum = (b_ih + b_hh).astype(np.float32)
    brz = np.zeros((128, 4), np.float32)
    brz[:, 0] = bsum[0:128]
    brz[:, 1] = bsum[128:256]
    brz[:, 2] = -bsum[256:384]
    brz[:, 3] = -bsum[384:512]
    bn = np.zeros((128, 2), np.float32)
    bn[:, 0] = b_ih[512:640]
    bn[:, 1] = b_ih[640:768]
    # ps_n bias matmul: out[f, c] = sum_p bconst[p, f] * sel[p, c]
    bconst = np.zeros((128, 128), np.float32)
    bconst[0, :] = b_hh[512:640]
    bconst[1, :] = b_hh[640:768]
    bconst = bconst.astype(BF16NP)
    sel = np.zeros((128, SCOLS), np.float32)
    sel[0, 0:GCOLS] = 1.0
    sel[1, GCOLS:SCOLS] = 1.0
    sel = sel.astype(BF16NP)
    ident = np.eye(128, dtype=np.float32)

    in_maps = []
    for c in range(NCORES):
        bs = slice(c * BS, (c + 1) * BS)
        xc = np.asarray(x[bs, :T]).transpose(2, 1, 0)       # [I, T, BS]
        xt = np.zeros((2, 128, xpad * BS), BF16NP)
        xt[:, :, :T * BS] = xc.reshape(2, 128, T * BS).astype(BF16NP)
        mwc = mwneg[bs].T                                    # [T, BS]
        mwt = np.zeros((1, xpad * SCOLS), np.float32)
        mwt[0, :T * SCOLS] = np.concatenate([mwc, mwc], axis=1).reshape(-1)
        in_maps.append({
            "xt": xt, "mw": mwt, "whT": whT, "whTn": whTn, "wiT": wiT,
            "brz": brz, "bn": bn, "bconst": bconst, "sel": sel, "ident": ident,
        })
    return in_maps


def assemble_out(results):
    out = np.zeros((B, H), np.float32)
    for c, res in enumerate(results):
        ho = res["hout"]                      # [128, 16]
        for k in range(2):
            out[c * BS:(c + 1) * BS, k * 128:(k + 1) * 128] = \
                ho[:, k * GCOLS:(k + 1) * GCOLS].T
    return out


def kernel(x, att_weights, lengths, W_ih, W_hh, b_ih, b_hh):
    x = np.asarray(x)
    lengths = np.asarray(lengths)
    T_run = plan_T(lengths)
    in_maps = host_prep(np.asarray(x), np.asarray(att_weights),
                        lengths, np.asarray(W_ih),
                        np.asarray(W_hh), np.asarray(b_ih),
                        np.asarray(b_hh), T_run)
    nc = build_nc(T_run)
    res = None
    for attempt in range(3):
        try:
            res = run_bass_kernel_spmd(nc, in_maps, core_ids=list(range(NCORES)))
            break
        except Exception:
            if attempt == 2:
                raise
    return assemble_out(res.results)
